# revision 29
# baseline (speedup 1.0000x reference)
"""Trainium2 Bass kernel for DenseCapsule dynamic routing (3 iterations).

Problem: x[128,2048,8] f32, weight[16,2048,16,8] f32 -> out[128,16,16] f32.
  x_hat = einsum('oide,bie->boid', W, x); 3 routing iterations
  (softmax over o, c-weighted i-sum, squash, agreement update).

Strategy (8 NeuronCores, shard in_num_caps I=2048 -> 256 per core):
  x_hat is never materialized; everything factors through W:
    s = (c*x) @ W        (PE, contraction over (i,e))
    u = v . W            (PE, contraction over d)
    l = sum_e x*u        (DVE/Pool elementwise + e-tree)
  Cross-core: AllReduce of partial s after iters 1 and 2 (split into
  o-halves so each AR overlaps compute of the other half); the final
  iteration's partial s is returned per-core and the host does the
  gather-sum + final squash.

v2 layout: s-matmuls are FLIPPED (stationary = W [il,16d] slices, moving
  = xc [il,b]) with 4-way PE column tiling -> psum s lands as [d, b] per o
  in "quad" tiles [(4o x 32dd), b] (o at 32-aligned offsets, d in rows
  0..15 of each 32-block). That is exactly the rhs layout the u-matmul
  needs, so the per-iteration v transposes disappear. Squash runs in this
  layout via small ones-pattern matmuls (nrm2 + scale broadcast).

  A tiny warmup AllReduce is issued at t=0 so the one-time collectives
  barrier/launch-skew cost overlaps the input DMAs and iter-1 compute.
"""

import sys

for _p in ("/opt/trn_rl_repo", "/root/.axon_site/_ro/trn_rl_repo"):
    if _p not in sys.path:
        sys.path.insert(0, _p)

import numpy as np
import ml_dtypes

import concourse.bass as bass
import concourse.bacc as bacc
import concourse.mybir as mybir
import concourse.tile as tile
from concourse.bass_utils import run_bass_kernel_spmd

F32 = mybir.dt.float32
BF16 = mybir.dt.bfloat16
NPBF16 = ml_dtypes.bfloat16

N_CORES = 8
B = 128          # batch
I_FULL = 2048    # in caps
IC = 256         # in caps per core
IL = 128         # partition dim of i
IH = IC // IL    # 2
E = 8            # in cap dim
O = 16           # out caps
D = 16           # out cap dim
NQ = 4           # o-quads; o = 4*q + g, g = col group

# engine split for the e-reduction tree (per o): True -> GpSimd (Pool)
POOL_TREE = [bool(o % 2) for o in range(O)]
# engine split for xc: True -> Pool
POOL_XC = [o % 4 == 3 for o in range(O)]

_CACHE = {}


def _emit_ul(nc, pools, v_q, l_buf, delta_buf, wdt, xbf, itr):
    """u = v.W (PE row-tiled) -> evac (ACT) -> xu = x*u (DVE) ->
    e-tree (DVE/Pool) -> l (or delta for iter 3)."""
    pool, psum_u, seq = pools
    for o in range(O):
        q, g = o // 4, o % 4
        for ih in range(IH):
            u_ps = psum_u.tile([IL, E * B], F32, tag="u")
            for e in range(E):
                lhsT = wdt[:, :].rearrange(
                    "p (q ih e il) -> p q ih e il", q=NQ, ih=IH, e=E
                )[32 * g:32 * (g + 1), q, ih, e, :]
                rhs = v_q[q][32 * g:32 * (g + 1), :]
                nc.tensor.matmul(
                    u_ps[:, e * B:(e + 1) * B], lhsT, rhs,
                    start=True, stop=True, tile_position=(32 * g, 0),
                )
            u_sb = pool.tile([IL, E * B], BF16, tag="usb")
            nc.scalar.copy(u_sb[:, :], u_ps[:, :])
            xu = pool.tile([IL, E * B], BF16, tag="xu")
            nc.vector.tensor_tensor(
                xu[:, :],
                xbf[:, :].rearrange("p (ih x) -> p ih x", ih=IH)[:, ih],
                u_sb[:, :], op=mybir.AluOpType.mult,
            )
            eng = nc.gpsimd if POOL_TREE[o] else nc.vector
            r1 = pool.tile([IL, 4 * B], BF16, tag="r1")
            eng.tensor_tensor(r1[:, :], xu[:, 0:4 * B], xu[:, 4 * B:8 * B],
                              op=mybir.AluOpType.add)
            r2 = pool.tile([IL, 2 * B], BF16, tag="r2")
            eng.tensor_tensor(r2[:, :], r1[:, 0:2 * B], r1[:, 2 * B:4 * B],
                              op=mybir.AluOpType.add)
            dst_buf = l_buf if itr == 2 else delta_buf
            dst = dst_buf[:, :].rearrange(
                "p (o ih b) -> p o ih b", o=O, ih=IH)[:, o, ih]
            eng.tensor_tensor(dst, r2[:, 0:B], r2[:, B:2 * B],
                              op=mybir.AluOpType.add)
    if itr == 3:
        HB = O * IH * B // 2
        nc.vector.tensor_add(l_buf[:, 0:HB], l_buf[:, 0:HB],
                             delta_buf[:, 0:HB])
        nc.vector.tensor_add(l_buf[:, HB:2 * HB], l_buf[:, HB:2 * HB],
                             delta_buf[:, HB:2 * HB])


def _emit_softmax_xc_s(nc, pools, l_buf, xbf, wbf, s_q, itr):
    """exp (ACT), Z (DVE), 1/Z, xp = x/Z, then per-o xc = exp*xp and the
    flipped s-matmuls (stationary = W [il,16], moving = xc [il,b]) into
    per-quad psum tiles s_q[q][(4g x 32dd), b] (one bank each)."""
    pool, psum_u, seq = pools
    exp_buf = seq.tile([IL, O * IH * B], BF16, tag="exp")
    HALF = 8 * IH * B
    nc.scalar.activation(
        exp_buf[:, 0:HALF], l_buf[:, 0:HALF],
        mybir.ActivationFunctionType.Exp)
    za1 = seq.tile([IL, 4 * IH * B], F32, tag="za1")
    nc.vector.tensor_add(za1[:, :], exp_buf[:, 0:HALF // 2],
                         exp_buf[:, HALF // 2:HALF])
    za2 = seq.tile([IL, 2 * IH * B], F32, tag="za2")
    nc.vector.tensor_add(za2[:, :], za1[:, 0:2 * IH * B],
                         za1[:, 2 * IH * B:4 * IH * B])
    za3 = seq.tile([IL, IH * B], F32, tag="za3")
    nc.vector.tensor_add(za3[:, :], za2[:, 0:IH * B],
                         za2[:, IH * B:2 * IH * B])
    nc.scalar.activation(
        exp_buf[:, HALF:2 * HALF], l_buf[:, HALF:2 * HALF],
        mybir.ActivationFunctionType.Exp)
    zb1 = seq.tile([IL, 4 * IH * B], F32, tag="zb1")
    nc.vector.tensor_add(zb1[:, :], exp_buf[:, HALF:HALF + HALF // 2],
                         exp_buf[:, HALF + HALF // 2:2 * HALF])
    zb2 = seq.tile([IL, 2 * IH * B], F32, tag="zb2")
    nc.vector.tensor_add(zb2[:, :], zb1[:, 0:2 * IH * B],
                         zb1[:, 2 * IH * B:4 * IH * B])
    zb3 = seq.tile([IL, IH * B], F32, tag="zb3")
    nc.vector.tensor_add(zb3[:, :], zb2[:, 0:IH * B],
                         zb2[:, IH * B:2 * IH * B])
    zbuf = seq.tile([IL, IH * B], F32, tag="z")
    nc.vector.tensor_add(zbuf[:, :], za3[:, :], zb3[:, :])
    rz = seq.tile([IL, IH * B], F32, tag="rz")
    nc.vector.reciprocal(rz[:, :], zbuf[:, :])
    rzbf = seq.tile([IL, IH * B], BF16, tag="rzbf")
    nc.vector.tensor_copy(rzbf[:, :], rz[:, :])
    xp = seq.tile([IL, IH * E * B], BF16, tag="xp")
    nc.vector.tensor_tensor(
        xp[:, :].rearrange("p (ih e b) -> p ih e b", ih=IH, e=E),
        xbf[:, :].rearrange("p (ih e b) -> p ih e b", ih=IH, e=E),
        rzbf[:, :].rearrange("p (ih b) -> p ih b", ih=IH)
        .unsqueeze(2).broadcast_to((IL, IH, E, B)),
        op=mybir.AluOpType.mult,
    )
    for q in range(NQ):
        # accumulate-onto-zero: col-tiled groups can't use start/stop in a
        # shared bank (whole-bank lazy zeroing), so zero explicitly.
        nc.scalar.memzero(s_q[q][:, 0:B])
        xcs = []
        for g in range(4):
            o = 4 * q + g
            xc = pool.tile([IL, IH * E * B], BF16, tag=f"xc{g}")
            eng = nc.gpsimd if POOL_XC[o] else nc.vector
            eng.tensor_tensor(
                xc[:, :].rearrange("p (ih e b) -> p ih e b", ih=IH, e=E),
                exp_buf[:, :].rearrange(
                    "p (o ih b) -> p o ih b", o=O, ih=IH)[:, o]
                .unsqueeze(2).broadcast_to((IL, IH, E, B)),
                xp[:, :].rearrange("p (ih e b) -> p ih e b", ih=IH, e=E),
                op=mybir.AluOpType.mult,
            )
            xcs.append(xc)
        kt = 0
        for ih in range(IH):
            for e in range(E):
                for g in range(4):
                    o = 4 * q + g
                    lhsT = wbf[:, :].rearrange(
                        "p (ih e o d) -> p ih e o d", ih=IH, e=E, o=O
                    )[:, ih, e, o, :]
                    rhs = xcs[g][:, :].rearrange(
                        "p (ih e b) -> p ih e b", ih=IH, e=E)[:, ih, e, :]
                    nc.tensor.matmul(
                        s_q[q][32 * g:32 * g + D, 0:B],
                        lhsT, rhs,
                        start=False, stop=False,
                        tile_position=(0, 32 * g),
                        skip_group_check=True,
                    )
                kt += 1


def _emit_s_export(nc, spool, s_q, dram_half, half, scale):
    """Evacuate quads of one half (ACT, optional scale) and DMA the valid
    (o,d) rows to the packed DRAM blob [128=(2q,4g,16d), B] slice."""
    for qq in range(2):
        q = 2 * half + qq
        s_sb = spool.tile([128, B], F32, tag=f"ssb{qq}")
        src = s_q[q][:, 0:B]
        if scale != 1.0:
            nc.scalar.mul(s_sb[:, :], src, scale)
        else:
            nc.scalar.copy(s_sb[:, :], src)
        for g in range(4):
            nc.sync.dma_start(
                out=dram_half[64 * qq + 16 * g:64 * qq + 16 * (g + 1), :],
                in_=s_sb[32 * g:32 * g + D, :])


def _emit_squash_half(nc, pools_sq, cc_out_half, half, sfull_q, v_q,
                      ones_n, ones_bc, m_ps, tag):
    """DMA the AR result for one o-half back into quad tiles, then squash:
    nrm2 via one ones-matmul (-> m_ps[0:4, 2B:4B]), scale = sqrt(n)/(1+n)
    on [4, 2B], broadcast over (o,d) rows via one ones_bc matmul
    (-> m_ps[:, 0:2B]), v = s * scale_bc (bf16 quads)."""
    spool, _ = pools_sq
    sq = spool.tile([128, 2 * B], BF16, tag="sq")
    for qq in range(2):
        q = 2 * half + qq
        for g in range(4):
            nc.sync.dma_start(
                out=sfull_q[q][32 * g:32 * g + D, :],
                in_=cc_out_half[64 * qq + 16 * g:64 * qq + 16 * (g + 1), :])
        nc.scalar.square(sq[:, qq * B:(qq + 1) * B], sfull_q[q][:, :])
    nc.tensor.matmul(
        m_ps[0:4, 2 * B:4 * B], ones_n[:, :], sq[:, :],
        start=True, stop=True,
    )
    # scale chain on [4, 2B] f32
    nsl = m_ps[0:4, 2 * B:4 * B]
    qrt = spool.tile([4, 2 * B], F32, tag="qrt")
    nc.scalar.sqrt(qrt[:, :], nsl)
    t1 = spool.tile([4, 2 * B], F32, tag="t1")
    nc.vector.tensor_scalar_add(t1[:, :], nsl, 1.0)
    rt = spool.tile([4, 2 * B], F32, tag="rt")
    nc.vector.reciprocal(rt[:, :], t1[:, :])
    scale = spool.tile([4, 2 * B], BF16, tag="scale")
    nc.vector.tensor_mul(scale[:, :], qrt[:, :], rt[:, :])
    nc.tensor.matmul(
        m_ps[:, 0:2 * B], ones_bc[:, :], scale[:, :],
        start=True, stop=True,
    )
    for qq in range(2):
        q = 2 * half + qq
        scbc = spool.tile([128, B], BF16, tag=f"scbc{qq}")
        nc.scalar.copy(scbc[:, :], m_ps[:, qq * B:(qq + 1) * B])
        nc.vector.tensor_tensor(
            v_q[q][:, :], sfull_q[q][:, :], scbc[:, :],
            op=mybir.AluOpType.mult,
        )


def build():
    nc = bacc.Bacc("TRN2", target_bir_lowering=False, debug=False,
                   enable_asserts=True, num_devices=N_CORES)

    xbf_d = nc.dram_tensor("xbf", [IL, IH * E * B], BF16,
                           kind="ExternalInput").ap()
    wbf_d = nc.dram_tensor("wbf", [IL, IH * E * O * D], BF16,
                           kind="ExternalInput").ap()
    wdt_d = nc.dram_tensor("wdt", [128, NQ * IH * E * IL], BF16,
                           kind="ExternalInput").ap()
    ones_n_d = nc.dram_tensor("ones_n", [128, 4], BF16,
                              kind="ExternalInput").ap()
    ones_bc_d = nc.dram_tensor("ones_bc", [4, 128], BF16,
                               kind="ExternalInput").ap()
    sp_out = nc.dram_tensor("sp", [O * D, B], F32, kind="ExternalOutput").ap()

    warm_in = nc.dram_tensor("warm_in", [1, 8], F32)
    warm_out = nc.dram_tensor("warm_out", [1, 8], F32, addr_space="Shared")
    cc1_in = nc.dram_tensor("cc1_in", [O * D, B], F32)
    cc1_out = nc.dram_tensor("cc1_out", [O * D, B], F32, addr_space="Shared")
    cc2_in = nc.dram_tensor("cc2_in", [O * D, B], F32)
    cc2_out = nc.dram_tensor("cc2_out", [O * D, B], F32, addr_space="Shared")

    rg = [list(range(N_CORES))]

    with tile.TileContext(nc) as tc:
        with (
            tc.tile_pool(name="const", bufs=1) as cpool,
            tc.tile_pool(name="work", bufs=4) as pool,
            tc.tile_pool(name="spool", bufs=2) as spool,
            tc.tile_pool(name="seq", bufs=1) as seq_pool,
            tc.tile_pool(name="psum_u", bufs=2, space="PSUM") as psum_u,
            tc.tile_pool(name="psum_g", bufs=4, space="PSUM") as psum_g,
        ):
            # warmup collective: absorbs the one-time collectives
            # barrier / launch-skew behind input DMA + s1 compute.
            warm_sb = cpool.tile([1, 8], F32)
            nc.vector.memset(warm_sb[:, :], 0.0)
            nc.sync.dma_start(out=warm_in[:], in_=warm_sb[:, :])
            nc.gpsimd.collective_compute(
                "AllReduce", mybir.AluOpType.add, replica_groups=rg,
                ins=[warm_in[:]], outs=[warm_out[:]],
            )

            # ---- load inputs ----
            xbf = cpool.tile([IL, IH * E * B], BF16)
            nc.sync.dma_start(out=xbf[:, :], in_=xbf_d)
            wbf = cpool.tile([IL, IH * E * O * D], BF16)
            nc.sync.dma_start(out=wbf[:, :], in_=wbf_d)
            ones_n = cpool.tile([128, 4], BF16)
            nc.sync.dma_start(out=ones_n[:, :], in_=ones_n_d)
            ones_bc = cpool.tile([4, 128], BF16)
            nc.sync.dma_start(out=ones_bc[:, :], in_=ones_bc_d)
            wdt = cpool.tile([128, NQ * IH * E * IL], BF16)
            nc.sync.dma_start(out=wdt[:, :], in_=wdt_d)

            l_buf = cpool.tile([IL, O * IH * B], BF16)
            delta_buf = cpool.tile([IL, O * IH * B], BF16)
            v_q = [cpool.tile([128, B], BF16, tag=f"v{q}", name=f"v{q}")
                   for q in range(NQ)]
            sfull_q = [cpool.tile([128, B], F32, tag=f"sf{q}", name=f"sf{q}")
                       for q in range(NQ)]
            for q in range(NQ):
                nc.vector.memset(v_q[q][:, :], 0.0)
                nc.gpsimd.memset(sfull_q[q][:, :], 0.0)

            pools = (pool, psum_u, seq_pool)
            pools_sq = (spool, psum_g)

            def s_quad_tiles(stage):
                return [psum_g.tile([128, 512], F32, tag="g",
                                    name=f"s{stage}q{q}") for q in range(NQ)]

            # ---- iteration 1: uniform c -> s1 = (1/16) x @ W ----
            _sid_s1, _ = nc.enter_named_scope("s1", False)
            s_q1 = s_quad_tiles(1)
            for q in range(NQ):
                nc.scalar.memzero(s_q1[q][:, 0:B])
                kt = 0
                for ih in range(IH):
                    for e in range(E):
                        rhs = xbf[:, :].rearrange(
                            "p (ih e b) -> p ih e b", ih=IH, e=E)[:, ih, e, :]
                        for g in range(4):
                            o = 4 * q + g
                            lhsT = wbf[:, :].rearrange(
                                "p (ih e o d) -> p ih e o d",
                                ih=IH, e=E, o=O)[:, ih, e, o, :]
                            nc.tensor.matmul(
                                s_q1[q][32 * g:32 * g + D, 0:B],
                                lhsT, rhs,
                                start=False, stop=False,
                                tile_position=(0, 32 * g),
                                skip_group_check=True,
                            )
                        kt += 1
            for half in range(2):
                _emit_s_export(nc, spool, s_q1,
                               cc1_in[128 * half:128 * (half + 1), :],
                               half, 1.0 / O)
            nc.leave_named_scope("s1", _sid_s1, False)

            _sid_ar1, _ = nc.enter_named_scope("ar1", False)
            for half in range(2):
                nc.gpsimd.collective_compute(
                    "AllReduce", mybir.AluOpType.add, replica_groups=rg,
                    ins=[cc1_in[128 * half:128 * (half + 1), :]],
                    outs=[cc1_out[128 * half:128 * (half + 1), :]],
                )
                m_ps = psum_g.tile([128, 512], F32, tag="g",
                                   name=f"m1h{half}")
                _emit_squash_half(
                    nc, pools_sq, cc1_out[128 * half:128 * (half + 1), :],
                    half, sfull_q, v_q, ones_n, ones_bc, m_ps, "1")
            nc.leave_named_scope("ar1", _sid_ar1, False)

            # ---- iteration 2 ----
            _sid_ul2, _ = nc.enter_named_scope("ul2", False)
            _emit_ul(nc, pools, v_q, l_buf, delta_buf, wdt, xbf, 2)
            nc.leave_named_scope("ul2", _sid_ul2, False)
            _sid_xcs2, _ = nc.enter_named_scope("xcs2", False)
            s_q2 = s_quad_tiles(2)
            _emit_softmax_xc_s(nc, pools, l_buf, xbf, wbf, s_q2, 2)
            for half in range(2):
                _emit_s_export(nc, spool, s_q2,
                               cc2_in[128 * half:128 * (half + 1), :],
                               half, 1.0)
            nc.leave_named_scope("xcs2", _sid_xcs2, False)

            _sid_ar2, _ = nc.enter_named_scope("ar2", False)
            for half in range(2):
                nc.gpsimd.collective_compute(
                    "AllReduce", mybir.AluOpType.add, replica_groups=rg,
                    ins=[cc2_in[128 * half:128 * (half + 1), :]],
                    outs=[cc2_out[128 * half:128 * (half + 1), :]],
                )
                m_ps = psum_g.tile([128, 512], F32, tag="g",
                                   name=f"m2h{half}")
                _emit_squash_half(
                    nc, pools_sq, cc2_out[128 * half:128 * (half + 1), :],
                    half, sfull_q, v_q, ones_n, ones_bc, m_ps, "2")
            nc.leave_named_scope("ar2", _sid_ar2, False)

            # ---- iteration 3 (final: partial s3 out, host finishes) ----
            _sid_ul3, _ = nc.enter_named_scope("ul3", False)
            _emit_ul(nc, pools, v_q, l_buf, delta_buf, wdt, xbf, 3)
            nc.leave_named_scope("ul3", _sid_ul3, False)
            _sid_xcs3, _ = nc.enter_named_scope("xcs3", False)
            s_q3 = s_quad_tiles(3)
            _emit_softmax_xc_s(nc, pools, l_buf, xbf, wbf, s_q3, 3)
            for half in range(2):
                _emit_s_export(nc, spool, s_q3,
                               sp_out[128 * half:128 * (half + 1), :],
                               half, 1.0)
            nc.leave_named_scope("xcs3", _sid_xcs3, False)

    nc.compile()
    return nc


def _host_prep(x, weight):
    """Build the per-core input maps (free host-side rearrangement)."""
    in_maps = []
    ones_n = np.zeros((128, 4), dtype=np.float32)
    ones_bc = np.zeros((4, 128), dtype=np.float32)
    for g in range(4):
        ones_n[32 * g:32 * g + D, g] = 1.0
        ones_bc[g, 32 * g:32 * g + D] = 1.0
    for c in range(N_CORES):
        x_c = x[:, c * IC:(c + 1) * IC, :]          # [B, 256, E]
        w_c = weight[:, c * IC:(c + 1) * IC, :, :]  # [O, 256, D, E]

        # xt [il, (ih, e, b)]
        xr = x_c.reshape(B, IH, IL, E)              # b, ih, il, e
        xt = np.ascontiguousarray(
            xr.transpose(2, 1, 3, 0)                # il, ih, e, b
        ).reshape(IL, IH * E * B)

        # wbf [il, (ih, e, o, d)]
        wr = w_c.reshape(O, IH, IL, D, E)           # o, ih, il, d, e
        w_f = np.ascontiguousarray(
            wr.transpose(2, 1, 4, 0, 3)             # il, ih, e, o, d
        ).reshape(IL, IH * E * O * D)

        # wdt [(g, dd=32), (q, ih, e, il)] with o = 4q+g, dd>=16 zero
        wq = wr.reshape(NQ, 4, IH, IL, D, E)        # q, g, ih, il, d, e
        wdtv = np.zeros((4, 32, NQ, IH, E, IL), dtype=np.float32)
        wdtv[:, :D] = wq.transpose(1, 4, 0, 2, 5, 3)  # g, d, q, ih, e, il
        wdt = wdtv.reshape(128, NQ * IH * E * IL)

        in_maps.append({
            "xbf": xt.astype(NPBF16),
            "wbf": w_f.astype(NPBF16),
            "wdt": wdt.astype(NPBF16),
            "ones_n": ones_n.astype(NPBF16),
            "ones_bc": ones_bc.astype(NPBF16),
        })
    return in_maps


def _host_finish(partials):
    """Sum the 8 per-core partial s3 tensors ([O*D, B] each), final
    squash (the unshard)."""
    s = np.zeros((O * D, B), dtype=np.float64)
    for p in partials:
        s += p.astype(np.float64)
    s = s.reshape(O, D, B).transpose(2, 0, 1)       # [B, O, D]
    n2 = (s * s).sum(axis=-1, keepdims=True)
    n = np.sqrt(n2)
    v = (n2 / (1.0 + n2) / (n + 1e-8)) * s
    return v.astype(np.float32)


def kernel(x, weight, _trace=False):
    x = np.asarray(x, dtype=np.float32)
    weight = np.asarray(weight, dtype=np.float32)
    if "nc" not in _CACHE:
        _CACHE["nc"] = build()
    nc = _CACHE["nc"]
    in_maps = _host_prep(x, weight)
    res = run_bass_kernel_spmd(
        nc, in_maps, core_ids=list(range(N_CORES)), trace=_trace
    )
    out = _host_finish([res.results[c]["sp"] for c in range(N_CORES)])
    if _trace:
        _CACHE["last_result"] = res
    return out


if __name__ == "__main__":
    rng = np.random.default_rng(0)
    x = rng.standard_normal((B, I_FULL, E)).astype(np.float32)
    w = (0.01 * rng.standard_normal((O, I_FULL, D, E))).astype(np.float32)
    out = kernel(x, w)
    print("out", out.shape, out.dtype, np.abs(out).max())


# revision 38
# speedup vs baseline: 1.0767x; 1.0767x over previous
"""Trainium2 Bass kernel for DenseCapsule dynamic routing (3 iterations).

Problem: x[128,2048,8] f32, weight[16,2048,16,8] f32 -> out[128,16,16] f32.
  x_hat = einsum('oide,bie->boid', W, x); 3 routing iterations
  (softmax over o, c-weighted i-sum, squash, agreement update).

Strategy (8 NeuronCores, shard in_num_caps I=2048 -> 256 per core):
  x_hat is never materialized; everything factors through W:
    s = (c*x) @ W        (PE, contraction over (i,e))
    u = v . W            (PE, contraction over d)
    l = sum_e x*u        (DVE/Pool elementwise + e-tree)
  Cross-core: AllReduce of partial s after iters 1 and 2 (split into
  o-halves so each AR overlaps compute of the other half); the final
  iteration's partial s is returned per-core and the host does the
  gather-sum + final squash.

v2 layout: s-matmuls are FLIPPED (stationary = W [il,16d] slices, moving
  = xc [il,b]) with 4-way PE column tiling -> psum s lands as [d, b] per o
  in "quad" tiles [(4o x 32dd), b] (o at 32-aligned offsets, d in rows
  0..15 of each 32-block). That is exactly the rhs layout the u-matmul
  needs, so the per-iteration v transposes disappear. Squash runs in this
  layout via small ones-pattern matmuls (nrm2 + scale broadcast).

  A tiny warmup AllReduce is issued at t=0 so the one-time collectives
  barrier/launch-skew cost overlaps the input DMAs and iter-1 compute.
"""

import sys

for _p in ("/opt/trn_rl_repo", "/root/.axon_site/_ro/trn_rl_repo"):
    if _p not in sys.path:
        sys.path.insert(0, _p)

import numpy as np
import ml_dtypes

import concourse.bass as bass
import concourse.bacc as bacc
import concourse.mybir as mybir
import concourse.tile as tile
from concourse.bass_utils import run_bass_kernel_spmd

F32 = mybir.dt.float32
BF16 = mybir.dt.bfloat16
NPBF16 = ml_dtypes.bfloat16

N_CORES = 8
B = 128          # batch
I_FULL = 2048    # in caps
IC = 256         # in caps per core
IL = 128         # partition dim of i
IH = IC // IL    # 2
E = 8            # in cap dim
O = 16           # out caps
D = 16           # out cap dim
NQ = 4           # o-quads; o = 4*q + g, g = col group

_CACHE = {}


def _emit_ul(nc, pools, v_q, l_buf, delta_buf, wdt, xbf, itr):
    """u = v.W (PE row-tiled, per-ih psum banks) -> evac (ACT) ->
    xu = x*u (DVE, both ih merged) -> e-tree (DVE) -> l (or delta)."""
    pool, psum_u, seq = pools
    for o in range(O):
        q, g = o // 4, o % 4
        u_sb = pool.tile([IL, IH * E * B], BF16, tag="usb")
        for ih in range(IH):
            u_ps = psum_u.tile([IL, E * B], F32, tag="u")
            for e in range(E):
                lhsT = wdt[:, :].rearrange(
                    "p (q ih e il) -> p q ih e il", q=NQ, ih=IH, e=E
                )[32 * g:32 * (g + 1), q, ih, e, :]
                rhs = v_q[q][32 * g:32 * (g + 1), :]
                nc.tensor.matmul(
                    u_ps[:, e * B:(e + 1) * B], lhsT, rhs,
                    start=True, stop=True, tile_position=(32 * g, 0),
                )
            nc.scalar.copy(
                u_sb[:, ih * E * B:(ih + 1) * E * B], u_ps[:, :])
        xu = pool.tile([IL, IH * E * B], BF16, tag="xu")
        nc.vector.tensor_tensor(
            xu[:, :], xbf[:, :], u_sb[:, :], op=mybir.AluOpType.mult,
        )
        # e-tree within each ih block: 8 -> 4 -> 2 -> 1
        xu4 = xu[:, :].rearrange("p (ih half eb) -> p ih half eb",
                                 ih=IH, half=2)
        r1 = pool.tile([IL, IH * 4 * B], BF16, tag="r1")
        r1v = r1[:, :].rearrange("p (ih eb) -> p ih eb", ih=IH)
        nc.vector.tensor_tensor(r1v, xu4[:, :, 0], xu4[:, :, 1],
                                op=mybir.AluOpType.add)
        r1h = r1[:, :].rearrange("p (ih half eb) -> p ih half eb",
                                 ih=IH, half=2)
        r2 = pool.tile([IL, IH * 2 * B], BF16, tag="r2")
        r2v = r2[:, :].rearrange("p (ih eb) -> p ih eb", ih=IH)
        nc.vector.tensor_tensor(r2v, r1h[:, :, 0], r1h[:, :, 1],
                                op=mybir.AluOpType.add)
        r2h = r2[:, :].rearrange("p (ih half b) -> p ih half b",
                                 ih=IH, half=2)
        dst_buf = l_buf if itr == 2 else delta_buf
        dst = dst_buf[:, :].rearrange(
            "p (o ih b) -> p o ih b", o=O, ih=IH)[:, o]
        nc.vector.tensor_tensor(dst, r2h[:, :, 0], r2h[:, :, 1],
                                op=mybir.AluOpType.add)
    if itr == 3:
        HB = O * IH * B // 2
        nc.vector.tensor_add(l_buf[:, 0:HB], l_buf[:, 0:HB],
                             delta_buf[:, 0:HB])
        nc.vector.tensor_add(l_buf[:, HB:2 * HB], l_buf[:, HB:2 * HB],
                             delta_buf[:, HB:2 * HB])


def _emit_softmax_xc_s(nc, pools, l_buf, xbf, wbf, s_q, itr):
    """exp (ACT), Z (DVE), 1/Z, xp = x/Z, then per-o xc = exp*xp and the
    flipped s-matmuls (stationary = W [il,16], moving = xc [il,b]) into
    per-quad psum tiles s_q[q][(4g x 32dd), b] (one bank each)."""
    pool, psum_u, seq = pools
    exp_buf = seq.tile([IL, O * IH * B], BF16, tag="exp")
    HALF = 8 * IH * B
    nc.scalar.activation(
        exp_buf[:, 0:HALF], l_buf[:, 0:HALF],
        mybir.ActivationFunctionType.Exp)
    lowp = nc.allow_low_precision(
        reason="softmax Z partial sums in bf16; rel budget 2e-2")
    lowp.__enter__()
    za1 = seq.tile([IL, 4 * IH * B], BF16, tag="za1")
    nc.vector.tensor_add(za1[:, :], exp_buf[:, 0:HALF // 2],
                         exp_buf[:, HALF // 2:HALF])
    za2 = seq.tile([IL, 2 * IH * B], BF16, tag="za2")
    nc.vector.tensor_add(za2[:, :], za1[:, 0:2 * IH * B],
                         za1[:, 2 * IH * B:4 * IH * B])
    za3 = seq.tile([IL, IH * B], BF16, tag="za3")
    nc.vector.tensor_add(za3[:, :], za2[:, 0:IH * B],
                         za2[:, IH * B:2 * IH * B])
    nc.scalar.activation(
        exp_buf[:, HALF:2 * HALF], l_buf[:, HALF:2 * HALF],
        mybir.ActivationFunctionType.Exp)
    zb1 = seq.tile([IL, 4 * IH * B], BF16, tag="zb1")
    nc.vector.tensor_add(zb1[:, :], exp_buf[:, HALF:HALF + HALF // 2],
                         exp_buf[:, HALF + HALF // 2:2 * HALF])
    zb2 = seq.tile([IL, 2 * IH * B], BF16, tag="zb2")
    nc.vector.tensor_add(zb2[:, :], zb1[:, 0:2 * IH * B],
                         zb1[:, 2 * IH * B:4 * IH * B])
    zb3 = seq.tile([IL, IH * B], BF16, tag="zb3")
    nc.vector.tensor_add(zb3[:, :], zb2[:, 0:IH * B],
                         zb2[:, IH * B:2 * IH * B])
    zbuf = seq.tile([IL, IH * B], F32, tag="z")
    nc.vector.tensor_add(zbuf[:, :], za3[:, :], zb3[:, :])
    rz = seq.tile([IL, IH * B], BF16, tag="rz")
    nc.vector.reciprocal(rz[:, :], zbuf[:, :])
    lowp.__exit__(None, None, None)
    xp = seq.tile([IL, IH * E * B], BF16, tag="xp")
    nc.vector.tensor_tensor(
        xp[:, :].rearrange("p (ih e b) -> p ih e b", ih=IH, e=E),
        xbf[:, :].rearrange("p (ih e b) -> p ih e b", ih=IH, e=E),
        rz[:, :].rearrange("p (ih b) -> p ih b", ih=IH)
        .unsqueeze(2).broadcast_to((IL, IH, E, B)),
        op=mybir.AluOpType.mult,
    )
    for q in range(NQ):
        # accumulate-onto-zero: col-tiled groups can't use start/stop in a
        # shared bank (whole-bank lazy zeroing), so zero explicitly.
        nc.scalar.memzero(s_q[q][:, 0:B])
        xcs = []
        for g in range(4):
            o = 4 * q + g
            xc = pool.tile([IL, IH * E * B], BF16, tag=f"xc{g}")
            nc.vector.tensor_tensor(
                xc[:, :].rearrange("p (ih e b) -> p ih e b", ih=IH, e=E),
                exp_buf[:, :].rearrange(
                    "p (o ih b) -> p o ih b", o=O, ih=IH)[:, o]
                .unsqueeze(2).broadcast_to((IL, IH, E, B)),
                xp[:, :].rearrange("p (ih e b) -> p ih e b", ih=IH, e=E),
                op=mybir.AluOpType.mult,
            )
            xcs.append(xc)
        kt = 0
        for ih in range(IH):
            for e in range(E):
                for g in range(4):
                    o = 4 * q + g
                    lhsT = wbf[:, :].rearrange(
                        "p (ih e o d) -> p ih e o d", ih=IH, e=E, o=O
                    )[:, ih, e, o, :]
                    rhs = xcs[g][:, :].rearrange(
                        "p (ih e b) -> p ih e b", ih=IH, e=E)[:, ih, e, :]
                    nc.tensor.matmul(
                        s_q[q][32 * g:32 * g + D, 0:B],
                        lhsT, rhs,
                        start=False, stop=False,
                        tile_position=(0, 32 * g),
                        skip_group_check=True,
                    )
                kt += 1


def _emit_s_export(nc, spool, s_q, dram_half, half, scale):
    """Evacuate quads of one half (ACT, optional scale) and DMA the valid
    (o,d) rows to the packed DRAM blob [128=(2q,4g,16d), B] slice."""
    for qq in range(2):
        q = 2 * half + qq
        s_sb = spool.tile([128, B], F32, tag=f"ssb{qq}")
        src = s_q[q][:, 0:B]
        if scale != 1.0:
            nc.scalar.mul(s_sb[:, :], src, scale)
        else:
            nc.scalar.copy(s_sb[:, :], src)
        for g in range(4):
            nc.sync.dma_start(
                out=dram_half[64 * qq + 16 * g:64 * qq + 16 * (g + 1), :],
                in_=s_sb[32 * g:32 * g + D, :])


def _emit_squash_half(nc, pools_sq, cc_out_half, half, sfull_q, v_q,
                      ones_n, ones_bc, m_ps, tag):
    """DMA the AR result for one o-half back into quad tiles, then squash:
    nrm2 via one ones-matmul (-> m_ps[0:4, 2B:4B]), scale = sqrt(n)/(1+n)
    on [4, 2B], broadcast over (o,d) rows via one ones_bc matmul
    (-> m_ps[:, 0:2B]), v = s * scale_bc (bf16 quads)."""
    spool, _ = pools_sq
    sq = spool.tile([128, 2 * B], BF16, tag="sq")
    for qq in range(2):
        q = 2 * half + qq
        for g in range(4):
            nc.sync.dma_start(
                out=sfull_q[q][32 * g:32 * g + D, :],
                in_=cc_out_half[64 * qq + 16 * g:64 * qq + 16 * (g + 1), :])
        nc.scalar.square(sq[:, qq * B:(qq + 1) * B], sfull_q[q][:, :])
    nc.tensor.matmul(
        m_ps[0:4, 2 * B:4 * B], ones_n[:, :], sq[:, :],
        start=True, stop=True,
    )
    # scale chain on [4, 2B] f32
    nsl = m_ps[0:4, 2 * B:4 * B]
    qrt = spool.tile([4, 2 * B], F32, tag="qrt")
    nc.scalar.sqrt(qrt[:, :], nsl)
    t1 = spool.tile([4, 2 * B], F32, tag="t1")
    nc.vector.tensor_scalar_add(t1[:, :], nsl, 1.0)
    rt = spool.tile([4, 2 * B], F32, tag="rt")
    nc.vector.reciprocal(rt[:, :], t1[:, :])
    scale = spool.tile([4, 2 * B], BF16, tag="scale")
    nc.vector.tensor_mul(scale[:, :], qrt[:, :], rt[:, :])
    nc.tensor.matmul(
        m_ps[:, 0:2 * B], ones_bc[:, :], scale[:, :],
        start=True, stop=True,
    )
    for qq in range(2):
        q = 2 * half + qq
        scbc = spool.tile([128, B], BF16, tag=f"scbc{qq}")
        nc.scalar.copy(scbc[:, :], m_ps[:, qq * B:(qq + 1) * B])
        nc.vector.tensor_tensor(
            v_q[q][:, :], sfull_q[q][:, :], scbc[:, :],
            op=mybir.AluOpType.mult,
        )


def build():
    nc = bacc.Bacc("TRN2", target_bir_lowering=False, debug=False,
                   enable_asserts=True, num_devices=N_CORES)

    xbf_d = nc.dram_tensor("xbf", [IL, IH * E * B], BF16,
                           kind="ExternalInput").ap()
    wbf_d = nc.dram_tensor("wbf", [IL, IH * E * O * D], BF16,
                           kind="ExternalInput").ap()
    wdt_d = nc.dram_tensor("wdt", [128, NQ * IH * E * IL], BF16,
                           kind="ExternalInput").ap()
    ones_n_d = nc.dram_tensor("ones_n", [128, 4], BF16,
                              kind="ExternalInput").ap()
    ones_bc_d = nc.dram_tensor("ones_bc", [4, 128], BF16,
                               kind="ExternalInput").ap()
    sp_out = nc.dram_tensor("sp", [O * D, B], F32, kind="ExternalOutput").ap()

    cc1_in = nc.dram_tensor("cc1_in", [O * D, B], F32)
    cc1_out = nc.dram_tensor("cc1_out", [O * D, B], F32, addr_space="Shared")
    cc2_in = nc.dram_tensor("cc2_in", [O * D, B], F32)
    cc2_out = nc.dram_tensor("cc2_out", [O * D, B], F32, addr_space="Shared")

    rg = [list(range(N_CORES))]

    with tile.TileContext(nc) as tc:
        with (
            tc.tile_pool(name="const", bufs=1) as cpool,
            tc.tile_pool(name="work", bufs=4) as pool,
            tc.tile_pool(name="spool", bufs=2) as spool,
            tc.tile_pool(name="seq", bufs=1) as seq_pool,
            tc.tile_pool(name="psum_u", bufs=2, space="PSUM") as psum_u,
            tc.tile_pool(name="psum_g", bufs=4, space="PSUM") as psum_g,
        ):
            # ---- load inputs ----
            xbf = cpool.tile([IL, IH * E * B], BF16)
            nc.sync.dma_start(out=xbf[:, :], in_=xbf_d)
            wbf = cpool.tile([IL, IH * E * O * D], BF16)
            nc.sync.dma_start(out=wbf[:, :], in_=wbf_d)
            ones_n = cpool.tile([128, 4], BF16)
            nc.sync.dma_start(out=ones_n[:, :], in_=ones_n_d)
            ones_bc = cpool.tile([4, 128], BF16)
            nc.sync.dma_start(out=ones_bc[:, :], in_=ones_bc_d)
            wdt = cpool.tile([128, NQ * IH * E * IL], BF16)
            nc.sync.dma_start(out=wdt[:, :], in_=wdt_d)

            l_buf = cpool.tile([IL, O * IH * B], BF16)
            delta_buf = cpool.tile([IL, O * IH * B], BF16)
            v_q = [cpool.tile([128, B], BF16, tag=f"v{q}", name=f"v{q}")
                   for q in range(NQ)]
            sfull_q = [cpool.tile([128, B], F32, tag=f"sf{q}", name=f"sf{q}")
                       for q in range(NQ)]
            for q in range(NQ):
                nc.vector.memset(v_q[q][:, :], 0.0)
                nc.gpsimd.memset(sfull_q[q][:, :], 0.0)

            pools = (pool, psum_u, seq_pool)
            pools_sq = (spool, psum_g)

            def s_quad_tiles(stage):
                return [psum_g.tile([128, 512], F32, tag="g",
                                    name=f"s{stage}q{q}") for q in range(NQ)]

            # ---- iteration 1: uniform c -> s1 = (1/16) x @ W ----
            _sid_s1, _ = nc.enter_named_scope("s1", False)
            s_q1 = s_quad_tiles(1)
            for q in range(NQ):
                nc.scalar.memzero(s_q1[q][:, 0:B])
                kt = 0
                for ih in range(IH):
                    for e in range(E):
                        rhs = xbf[:, :].rearrange(
                            "p (ih e b) -> p ih e b", ih=IH, e=E)[:, ih, e, :]
                        for g in range(4):
                            o = 4 * q + g
                            lhsT = wbf[:, :].rearrange(
                                "p (ih e o d) -> p ih e o d",
                                ih=IH, e=E, o=O)[:, ih, e, o, :]
                            nc.tensor.matmul(
                                s_q1[q][32 * g:32 * g + D, 0:B],
                                lhsT, rhs,
                                start=False, stop=False,
                                tile_position=(0, 32 * g),
                                skip_group_check=True,
                            )
                        kt += 1
            for half in range(2):
                _emit_s_export(nc, spool, s_q1,
                               cc1_in[128 * half:128 * (half + 1), :],
                               half, 1.0 / O)
            nc.leave_named_scope("s1", _sid_s1, False)

            _sid_ar1, _ = nc.enter_named_scope("ar1", False)
            nc.gpsimd.collective_compute(
                "AllReduce", mybir.AluOpType.add, replica_groups=rg,
                ins=[cc1_in[:]], outs=[cc1_out[:]],
            )
            for half in range(2):
                m_ps = psum_g.tile([128, 512], F32, tag="g",
                                   name=f"m1h{half}")
                _emit_squash_half(
                    nc, pools_sq, cc1_out[128 * half:128 * (half + 1), :],
                    half, sfull_q, v_q, ones_n, ones_bc, m_ps, "1")
            nc.leave_named_scope("ar1", _sid_ar1, False)

            # ---- iteration 2 ----
            _sid_ul2, _ = nc.enter_named_scope("ul2", False)
            _emit_ul(nc, pools, v_q, l_buf, delta_buf, wdt, xbf, 2)
            nc.leave_named_scope("ul2", _sid_ul2, False)
            _sid_xcs2, _ = nc.enter_named_scope("xcs2", False)
            s_q2 = s_quad_tiles(2)
            _emit_softmax_xc_s(nc, pools, l_buf, xbf, wbf, s_q2, 2)
            for half in range(2):
                _emit_s_export(nc, spool, s_q2,
                               cc2_in[128 * half:128 * (half + 1), :],
                               half, 1.0)
            nc.leave_named_scope("xcs2", _sid_xcs2, False)

            _sid_ar2, _ = nc.enter_named_scope("ar2", False)
            for half in range(2):
                nc.gpsimd.collective_compute(
                    "AllReduce", mybir.AluOpType.add, replica_groups=rg,
                    ins=[cc2_in[128 * half:128 * (half + 1), :]],
                    outs=[cc2_out[128 * half:128 * (half + 1), :]],
                )
                m_ps = psum_g.tile([128, 512], F32, tag="g",
                                   name=f"m2h{half}")
                _emit_squash_half(
                    nc, pools_sq, cc2_out[128 * half:128 * (half + 1), :],
                    half, sfull_q, v_q, ones_n, ones_bc, m_ps, "2")
            nc.leave_named_scope("ar2", _sid_ar2, False)

            # ---- iteration 3 (final: partial s3 out, host finishes) ----
            _sid_ul3, _ = nc.enter_named_scope("ul3", False)
            _emit_ul(nc, pools, v_q, l_buf, delta_buf, wdt, xbf, 3)
            nc.leave_named_scope("ul3", _sid_ul3, False)
            _sid_xcs3, _ = nc.enter_named_scope("xcs3", False)
            s_q3 = s_quad_tiles(3)
            _emit_softmax_xc_s(nc, pools, l_buf, xbf, wbf, s_q3, 3)
            for half in range(2):
                _emit_s_export(nc, spool, s_q3,
                               sp_out[128 * half:128 * (half + 1), :],
                               half, 1.0)
            nc.leave_named_scope("xcs3", _sid_xcs3, False)

    nc.compile()
    return nc


def _host_prep(x, weight):
    """Build the per-core input maps (free host-side rearrangement)."""
    in_maps = []
    ones_n = np.zeros((128, 4), dtype=np.float32)
    ones_bc = np.zeros((4, 128), dtype=np.float32)
    for g in range(4):
        ones_n[32 * g:32 * g + D, g] = 1.0
        ones_bc[g, 32 * g:32 * g + D] = 1.0
    for c in range(N_CORES):
        x_c = x[:, c * IC:(c + 1) * IC, :]          # [B, 256, E]
        w_c = weight[:, c * IC:(c + 1) * IC, :, :]  # [O, 256, D, E]

        # xt [il, (ih, e, b)]
        xr = x_c.reshape(B, IH, IL, E)              # b, ih, il, e
        xt = np.ascontiguousarray(
            xr.transpose(2, 1, 3, 0)                # il, ih, e, b
        ).reshape(IL, IH * E * B)

        # wbf [il, (ih, e, o, d)]
        wr = w_c.reshape(O, IH, IL, D, E)           # o, ih, il, d, e
        w_f = np.ascontiguousarray(
            wr.transpose(2, 1, 4, 0, 3)             # il, ih, e, o, d
        ).reshape(IL, IH * E * O * D)

        # wdt [(g, dd=32), (q, ih, e, il)] with o = 4q+g, dd>=16 zero
        wq = wr.reshape(NQ, 4, IH, IL, D, E)        # q, g, ih, il, d, e
        wdtv = np.zeros((4, 32, NQ, IH, E, IL), dtype=np.float32)
        wdtv[:, :D] = wq.transpose(1, 4, 0, 2, 5, 3)  # g, d, q, ih, e, il
        wdt = wdtv.reshape(128, NQ * IH * E * IL)

        in_maps.append({
            "xbf": xt.astype(NPBF16),
            "wbf": w_f.astype(NPBF16),
            "wdt": wdt.astype(NPBF16),
            "ones_n": ones_n.astype(NPBF16),
            "ones_bc": ones_bc.astype(NPBF16),
        })
    return in_maps


def _host_finish(partials):
    """Sum the 8 per-core partial s3 tensors ([O*D, B] each), final
    squash (the unshard)."""
    s = np.zeros((O * D, B), dtype=np.float64)
    for p in partials:
        s += p.astype(np.float64)
    s = s.reshape(O, D, B).transpose(2, 0, 1)       # [B, O, D]
    n2 = (s * s).sum(axis=-1, keepdims=True)
    n = np.sqrt(n2)
    v = (n2 / (1.0 + n2) / (n + 1e-8)) * s
    return v.astype(np.float32)


def kernel(x, weight, _trace=False):
    x = np.asarray(x, dtype=np.float32)
    weight = np.asarray(weight, dtype=np.float32)
    if "nc" not in _CACHE:
        _CACHE["nc"] = build()
    nc = _CACHE["nc"]
    in_maps = _host_prep(x, weight)
    res = run_bass_kernel_spmd(
        nc, in_maps, core_ids=list(range(N_CORES)), trace=_trace
    )
    out = _host_finish([res.results[c]["sp"] for c in range(N_CORES)])
    if _trace:
        _CACHE["last_result"] = res
    return out


if __name__ == "__main__":
    rng = np.random.default_rng(0)
    x = rng.standard_normal((B, I_FULL, E)).astype(np.float32)
    w = (0.01 * rng.standard_normal((O, I_FULL, D, E))).astype(np.float32)
    out = kernel(x, w)
    print("out", out.shape, out.dtype, np.abs(out).max())


# revision 40
# speedup vs baseline: 1.3484x; 1.2523x over previous
"""Trainium2 Bass kernel for DenseCapsule dynamic routing (3 iterations).

Problem: x[128,2048,8] f32, weight[16,2048,16,8] f32 -> out[128,16,16] f32.
  x_hat = einsum('oide,bie->boid', W, x); 3 routing iterations
  (softmax over o, c-weighted i-sum, squash, agreement update).

Strategy (8 NeuronCores, shard in_num_caps I=2048 -> 256 per core):
  x_hat is never materialized. Per iteration, everything factors through W:
    u = v . W (PE), l = sum_e x*u (DVE), softmax (ACT/DVE),
    xc = c*x (DVE), s = xc @ W (PE).

  v4: iteration 1's c is UNIFORM, so s1 = (1/16) sum_i xhat is fully
  input-determined. Every core loads the FULL W (bf16, host-rotated so its
  own I-slice sits in blocks 0..1) and computes the full s1 locally with
  one 128-matmul chain (overlapped with the W DMA stream) -> NO AllReduce
  for iteration 1. The collectives runtime's ~38us barrier + ~36us
  first-collective init run in the background (~21us in, time-based) and
  are fully amortized by the time AR2 (the only collective) triggers.
  AR2 is split into o-halves so squash/transpose/ul3 of the first half
  overlap the second half's AllReduce.

Layout conventions per core (SBUF partition dim first):
  i_local = ihf*128 + il over the ROTATED I axis (own slice = ihf 0..1)
  o = 4*h + g            (g in 0..3 selects a 32-partition group, h in 0..3)
  d padded to 32 rows (dd) for the u-matmul stationary operand.
"""

import sys

for _p in ("/opt/trn_rl_repo", "/root/.axon_site/_ro/trn_rl_repo"):
    if _p not in sys.path:
        sys.path.insert(0, _p)

import numpy as np
import ml_dtypes

import concourse.bass as bass
import concourse.bacc as bacc
import concourse.mybir as mybir
import concourse.tile as tile
from concourse.bass_utils import run_bass_kernel_spmd

F32 = mybir.dt.float32
BF16 = mybir.dt.bfloat16
NPBF16 = ml_dtypes.bfloat16

N_CORES = 8
B = 128          # batch
I_FULL = 2048    # in caps
IC = 256         # in caps per core
IL = 128         # partition dim of i
IH = IC // IL    # 2 (own slice blocks)
NIH = I_FULL // IL  # 16 (full-I blocks, rotated: 0..1 are "ours")
E = 8            # in cap dim
O = 16           # out caps
D = 16           # out cap dim
EPS = 1e-8

_CACHE = {}


def _emit_squash_half(nc, pool, sfull_h, vpad, psum_pool, vT, ident, hbase,
                      tag):
    """squash on one o-half: sfull_h [(b)=128, (2h,g,d)=128] f32 covering
    o = 4*hbase .. 4*hbase+7; writes v into vpad cols for h = hbase,
    hbase+1 and produces the two vT column blocks via PE transposes."""
    HO = 8  # o's in this half
    sq = pool.tile([B, HO * D], F32, tag=f"sq{tag}")
    nc.scalar.square(sq[:, :], sfull_h[:, :])
    nrm2 = pool.tile([B, HO], F32, tag=f"nrm2{tag}")
    nc.vector.reduce_sum(
        nrm2[:, :],
        sq[:, :].rearrange("p (o d) -> p o d", d=D),
        axis=mybir.AxisListType.X,
    )
    q = pool.tile([B, HO], F32, tag=f"q{tag}")
    nc.scalar.sqrt(q[:, :], nrm2[:, :])
    t1 = pool.tile([B, HO], F32, tag=f"t1{tag}")
    nc.vector.tensor_scalar_add(t1[:, :], nrm2[:, :], 1.0)
    t2 = pool.tile([B, HO], F32, tag=f"t2{tag}")
    nc.vector.tensor_scalar_add(t2[:, :], q[:, :], EPS)
    den = pool.tile([B, HO], F32, tag=f"den{tag}")
    nc.vector.tensor_mul(den[:, :], t1[:, :], t2[:, :])
    rden = pool.tile([B, HO], F32, tag=f"rden{tag}")
    nc.vector.reciprocal(rden[:, :], den[:, :])
    scale = pool.tile([B, HO], F32, tag=f"scale{tag}")
    nc.vector.tensor_mul(scale[:, :], nrm2[:, :], rden[:, :])
    # v = s * scale (broadcast over d) into vpad[(b), (hh, g, dd<16)]
    s_v = sfull_h[:, :].rearrange("p (hh g d) -> p hh g d", hh=2, g=4)
    scale_v = scale[:, :].rearrange("p (hh g) -> p hh g", hh=2).broadcast_to(
        (B, 2, 4, D)
    )
    vslice = vpad[:, :].rearrange(
        "p (h g dd) -> p h g dd", h=4, g=4)[:, hbase:hbase + 2, :, 0:D]
    nc.vector.tensor_tensor(vslice, s_v, scale_v, op=mybir.AluOpType.mult)
    for h in (hbase, hbase + 1):
        tp = psum_pool.tile([128, B], F32, tag="ps")
        in_slice = vpad[:, h * 128:(h + 1) * 128]
        nc.tensor.transpose(tp[:, :], in_slice, ident[:, :])
        nc.scalar.copy(vT[:, h * B:(h + 1) * B], tp[:, :])


def _emit_iteration_ul(nc, tc, pools, vT, l_buf, delta_buf, wdt, xbf, itr):
    """u = v.W (PE) -> evac (ACT) -> xu = x*u (DVE) -> e-reduction rounds
    (DVE) -> l (or delta for iter 3)."""
    pool, psum_pool, seq = pools
    for o in range(O):
        h, g = o // 4, o % 4
        u_ps = psum_pool.tile([IL, IH * E * B], F32, tag="ps")
        for ih in range(IH):
            for e in range(E):
                lhsT = wdt[:, :].rearrange(
                    "p (h ih e il) -> p h ih e il", h=4, ih=IH, e=E
                )[32 * g:32 * (g + 1), h, ih, e, :]
                rhs = vT[32 * g:32 * (g + 1), h * B:(h + 1) * B]
                nc.tensor.matmul(
                    u_ps[:, (ih * E + e) * B:(ih * E + e + 1) * B], lhsT, rhs,
                    start=True, stop=True, tile_position=(32 * g, 0),
                )
        u_sb = pool.tile([IL, IH * E * B], BF16, tag="u_sb")
        nc.scalar.copy(u_sb[:, :], u_ps[:, :])
        xu = pool.tile([IL, IH * E * B], BF16, tag="xu")
        nc.vector.tensor_tensor(
            xu[:, :], xbf, u_sb[:, :], op=mybir.AluOpType.mult,
        )
        xu4 = xu[:, :].rearrange("p (ih half eb) -> p ih half eb",
                                 ih=IH, half=2)
        r1 = pool.tile([IL, IH * 4 * B], BF16, tag="r1")
        r1v = r1[:, :].rearrange("p (ih eb) -> p ih eb", ih=IH)
        nc.vector.tensor_tensor(r1v, xu4[:, :, 0], xu4[:, :, 1],
                                op=mybir.AluOpType.add)
        r1h = r1[:, :].rearrange("p (ih half eb) -> p ih half eb",
                                 ih=IH, half=2)
        r2 = pool.tile([IL, IH * 2 * B], BF16, tag="r2")
        r2v = r2[:, :].rearrange("p (ih eb) -> p ih eb", ih=IH)
        nc.vector.tensor_tensor(r2v, r1h[:, :, 0], r1h[:, :, 1],
                                op=mybir.AluOpType.add)
        r2h = r2[:, :].rearrange("p (ih half b) -> p ih half b",
                                 ih=IH, half=2)
        dst_buf = l_buf if itr == 2 else delta_buf
        dst = dst_buf[:, :].rearrange(
            "p (o ih b) -> p o ih b", o=O, ih=IH
        )[:, o]
        nc.vector.tensor_tensor(dst, r2h[:, :, 0], r2h[:, :, 1],
                                op=mybir.AluOpType.add)
    if itr == 3:
        HB = O * IH * B // 2
        nc.vector.tensor_add(l_buf[:, 0:HB], l_buf[:, 0:HB],
                             delta_buf[:, 0:HB])
        nc.vector.tensor_add(l_buf[:, HB:], l_buf[:, HB:],
                             delta_buf[:, HB:])


def _emit_softmax_xc_s(nc, tc, pools, l_buf, xbf, wbf, s_ps, itr):
    """exp (ACT), Z (DVE bf16 trees), 1/Z, xp = x/Z, then per-o xc = exp*xp
    and the 16 accumulating s-matmuls into s_ps [(b), (o,d)=256]."""
    pool, psum_pool, seq = pools
    exp_buf = seq.tile([IL, O * IH * B], BF16, tag="exp")
    HALF = 8 * IH * B
    lowp = nc.allow_low_precision(
        reason="softmax Z partial sums in bf16; rel budget 2e-2")
    lowp.__enter__()
    nc.scalar.activation(
        exp_buf[:, 0:HALF], l_buf[:, 0:HALF],
        mybir.ActivationFunctionType.Exp)
    za1 = seq.tile([IL, 4 * IH * B], BF16, tag="za1")
    nc.vector.tensor_add(za1[:, :], exp_buf[:, 0:HALF // 2],
                         exp_buf[:, HALF // 2:HALF])
    za2 = seq.tile([IL, 2 * IH * B], BF16, tag="za2")
    nc.vector.tensor_add(za2[:, :], za1[:, 0:2 * IH * B],
                         za1[:, 2 * IH * B:4 * IH * B])
    za3 = seq.tile([IL, IH * B], BF16, tag="za3")
    nc.vector.tensor_add(za3[:, :], za2[:, 0:IH * B],
                         za2[:, IH * B:2 * IH * B])
    nc.scalar.activation(
        exp_buf[:, HALF:2 * HALF], l_buf[:, HALF:2 * HALF],
        mybir.ActivationFunctionType.Exp)
    zb1 = seq.tile([IL, 4 * IH * B], BF16, tag="zb1")
    nc.vector.tensor_add(zb1[:, :], exp_buf[:, HALF:HALF + HALF // 2],
                         exp_buf[:, HALF + HALF // 2:2 * HALF])
    zb2 = seq.tile([IL, 2 * IH * B], BF16, tag="zb2")
    nc.vector.tensor_add(zb2[:, :], zb1[:, 0:2 * IH * B],
                         zb1[:, 2 * IH * B:4 * IH * B])
    zb3 = seq.tile([IL, IH * B], BF16, tag="zb3")
    nc.vector.tensor_add(zb3[:, :], zb2[:, 0:IH * B],
                         zb2[:, IH * B:2 * IH * B])
    zbuf = seq.tile([IL, IH * B], F32, tag="z")
    nc.vector.tensor_add(zbuf[:, :], za3[:, :], zb3[:, :])
    rz = seq.tile([IL, IH * B], BF16, tag="rz")
    nc.vector.reciprocal(rz[:, :], zbuf[:, :])
    lowp.__exit__(None, None, None)
    xp = seq.tile([IL, IH * E * B], BF16, tag="xp")
    nc.vector.tensor_tensor(
        xp[:, :].rearrange("p (ih e b) -> p ih e b", ih=IH, e=E),
        xbf.rearrange("p (ih e b) -> p ih e b", ih=IH, e=E),
        rz[:, :].rearrange("p (ih b) -> p ih b", ih=IH)
        .unsqueeze(2).broadcast_to((IL, IH, E, B)),
        op=mybir.AluOpType.mult,
    )
    for o in range(O):
        xc = pool.tile([IL, IH * E * B], BF16, tag="xc")
        nc.vector.tensor_tensor(
            xc[:, :].rearrange("p (ih e b) -> p ih e b", ih=IH, e=E),
            exp_buf[:, :].rearrange("p (o ih b) -> p o ih b", o=O, ih=IH)[:, o]
            .unsqueeze(2).broadcast_to((IL, IH, E, B)),
            xp[:, :].rearrange("p (ih e b) -> p ih e b", ih=IH, e=E),
            op=mybir.AluOpType.mult,
        )
        n_k = IH * E
        kt = 0
        for ih in range(IH):
            for e in range(E):
                lhsT = xc[:, :].rearrange(
                    "p (ih e b) -> p ih e b", ih=IH, e=E
                )[:, ih, e, :]
                rhs = wbf.rearrange(
                    "p (ih e o d) -> p ih e (o d)", ih=IH, e=E, o=O
                )[:, ih, e, o * D:(o + 1) * D]
                nc.tensor.matmul(
                    s_ps[:, o * D:(o + 1) * D], lhsT, rhs,
                    start=(kt == 0), stop=(kt == n_k - 1),
                )
                kt += 1


def build():
    nc = bacc.Bacc("TRN2", target_bir_lowering=False, debug=False,
                   enable_asserts=True, num_devices=N_CORES)

    # per-core inputs (host pre-arranged + I-axis rotated; see kernel())
    xf_d = nc.dram_tensor("xf", [IL, NIH * E * B], BF16,
                          kind="ExternalInput").ap()
    wf_d = nc.dram_tensor("wf", [IL, NIH * E * O * D], BF16,
                          kind="ExternalInput").ap()
    wdt_d = nc.dram_tensor("wdt", [128, 4 * IH * E * IL], BF16,
                           kind="ExternalInput").ap()
    ident_d = nc.dram_tensor("ident", [128, 128], F32,
                             kind="ExternalInput").ap()
    sp_out = nc.dram_tensor("sp", [B, O * D], F32, kind="ExternalOutput").ap()

    cc_in = [nc.dram_tensor(f"cc{i}_in", [B, 8 * D], F32) for i in range(2)]
    cc_out = [nc.dram_tensor(f"cc{i}_out", [B, 8 * D], F32,
                             addr_space="Shared") for i in range(2)]

    rg = [list(range(N_CORES))]

    with tile.TileContext(nc) as tc:
        with (
            tc.tile_pool(name="const", bufs=1) as cpool,
            tc.tile_pool(name="work", bufs=3) as pool,
            tc.tile_pool(name="psum", bufs=2, space="PSUM") as psum_pool,
            tc.tile_pool(name="seq", bufs=1) as seq_pool,
        ):
            # ---- load inputs; wf in per-ihf chunks so s1 pipelines ----
            xf = cpool.tile([IL, NIH * E * B], BF16)
            nc.sync.dma_start(out=xf[:, :], in_=xf_d)
            ident = cpool.tile([128, 128], F32)
            nc.sync.dma_start(out=ident[:, :], in_=ident_d)
            wdt = cpool.tile([128, 4 * IH * E * IL], BF16)
            nc.sync.dma_start(out=wdt[:, :], in_=wdt_d)
            WCH = E * O * D  # wf chunk cols per ihf block
            wf = cpool.tile([IL, NIH * WCH], BF16)
            for ihf in range(NIH):
                nc.sync.dma_start(
                    out=wf[:, ihf * WCH:(ihf + 1) * WCH],
                    in_=wf_d.rearrange(
                        "p (f r) -> p f r", f=NIH)[:, ihf, :])

            # slice views (rotation put this core's I-slice at blocks 0..1)
            xbf = xf[:, 0:IH * E * B]
            wbf = wf[:, 0:IH * WCH]

            l_buf = cpool.tile([IL, O * IH * B], BF16)
            delta_buf = cpool.tile([IL, O * IH * B], BF16)
            vpad = cpool.tile([B, 4 * 4 * 32], F32)
            nc.vector.memset(vpad[:, :], 0.0)
            vT = cpool.tile([128, 4 * B], BF16)

            pools = (pool, psum_pool, seq_pool)

            # ---- iteration 1: full s1 = (1/16) x @ W locally, no AR ----
            _sid_s1, _ = nc.enter_named_scope("s1", False)
            s_ps1 = psum_pool.tile([B, O * D], F32, tag="ps")
            kt = 0
            for ihf in range(NIH):
                for e in range(E):
                    lhsT = xf[:, :].rearrange(
                        "p (f e b) -> p f e b", f=NIH, e=E)[:, ihf, e, :]
                    rhs = wf[:, :].rearrange(
                        "p (f e od) -> p f e od", f=NIH, e=E)[:, ihf, e, :]
                    nc.tensor.matmul(
                        s_ps1[:, :], lhsT, rhs,
                        start=(kt == 0), stop=(kt == NIH * E - 1),
                    )
                    kt += 1
            s_sb1 = cpool.tile([B, O * D], F32)
            nc.scalar.mul(s_sb1[:, :], s_ps1[:, :], 1.0 / O)
            nc.leave_named_scope("s1", _sid_s1, False)
            _sid_sq1, _ = nc.enter_named_scope("squash1", False)
            for hb in (0, 2):
                _emit_squash_half(nc, cpool, s_sb1[:, 128 * (hb // 2):
                                                   128 * (hb // 2 + 1)],
                                  vpad, psum_pool, vT, ident, hb, f"1{hb}")
            nc.leave_named_scope("squash1", _sid_sq1, False)

            # ---- iteration 2 ----
            _sid_ul2, _ = nc.enter_named_scope("ul2", False)
            _emit_iteration_ul(nc, tc, pools, vT, l_buf, delta_buf, wdt,
                               xbf, 2)
            nc.leave_named_scope("ul2", _sid_ul2, False)
            _sid_xcs2, _ = nc.enter_named_scope("xcs2", False)
            s_ps2 = psum_pool.tile([B, O * D], F32, tag="ps")
            _emit_softmax_xc_s(nc, tc, pools, l_buf, xbf, wbf, s_ps2, 2)
            s_sb2 = [cpool.tile([B, 8 * D], F32, tag=f"ssb{i}",
                                name=f"ssb{i}") for i in range(2)]
            for half in range(2):
                nc.scalar.copy(s_sb2[half][:, :],
                               s_ps2[:, 128 * half:128 * (half + 1)])
                nc.sync.dma_start(out=cc_in[half][:],
                                  in_=s_sb2[half][:, :])
            nc.leave_named_scope("xcs2", _sid_xcs2, False)

            _sid_ar2, _ = nc.enter_named_scope("ar2", False)
            sfull2 = [cpool.tile([B, 8 * D], F32, tag=f"sf{i}",
                                 name=f"sf{i}") for i in range(2)]
            for half in range(2):
                nc.gpsimd.collective_compute(
                    "AllReduce", mybir.AluOpType.add, replica_groups=rg,
                    ins=[cc_in[half][:]], outs=[cc_out[half][:]],
                )
                nc.sync.dma_start(out=sfull2[half][:, :],
                                  in_=cc_out[half][:])
                _emit_squash_half(nc, cpool, sfull2[half], vpad, psum_pool,
                                  vT, ident, 2 * half, f"2{half}")
            nc.leave_named_scope("ar2", _sid_ar2, False)

            # ---- iteration 3 (final: partial s3 out, host finishes) ----
            _sid_ul3, _ = nc.enter_named_scope("ul3", False)
            _emit_iteration_ul(nc, tc, pools, vT, l_buf, delta_buf, wdt,
                               xbf, 3)
            nc.leave_named_scope("ul3", _sid_ul3, False)
            _sid_xcs3, _ = nc.enter_named_scope("xcs3", False)
            s_ps3 = psum_pool.tile([B, O * D], F32, tag="ps")
            _emit_softmax_xc_s(nc, tc, pools, l_buf, xbf, wbf, s_ps3, 3)
            nc.leave_named_scope("xcs3", _sid_xcs3, False)
            sp_sb = cpool.tile([B, O * D], F32)
            nc.scalar.copy(sp_sb[:, :], s_ps3[:, :])
            nc.sync.dma_start(out=sp_out, in_=sp_sb[:, :])

    nc.compile()
    return nc


def _host_prep(x, weight):
    """Per-core input maps. The I axis is rotated per core so that the
    core's own slice occupies blocks 0..1 of the full tensors."""
    in_maps = []
    ident = np.eye(128, dtype=np.float32)
    for c in range(N_CORES):
        order = np.concatenate([
            np.arange(c * IC, (c + 1) * IC),
            np.arange(0, c * IC),
            np.arange((c + 1) * IC, I_FULL),
        ])
        x_r = x[:, order, :]          # [B, I, E]
        w_r = weight[:, order, :, :]  # [O, I, D, E]

        # xf [il, (ihf, e, b)]
        xr = x_r.reshape(B, NIH, IL, E)
        xf = np.ascontiguousarray(
            xr.transpose(2, 1, 3, 0)              # il, ihf, e, b
        ).reshape(IL, NIH * E * B)

        # wf [il, (ihf, e, o, d)]
        wr = w_r.reshape(O, NIH, IL, D, E)
        wfull = np.ascontiguousarray(
            wr.transpose(2, 1, 4, 0, 3)           # il, ihf, e, o, d
        ).reshape(IL, NIH * E * O * D)

        # wdt [(g, dd=32), (h, ih, e, il)] from own slice, o = 4h+g
        w_own = w_r[:, 0:IC]                      # [O, 256, D, E]
        wo = w_own.reshape(4, 4, IH, IL, D, E)    # h, g, ih, il, d, e
        wdtv = np.zeros((4, 32, 4, IH, E, IL), dtype=np.float32)
        wdtv[:, :D] = wo.transpose(1, 4, 0, 2, 5, 3)  # g, d, h, ih, e, il
        wdt = wdtv.reshape(128, 4 * IH * E * IL)

        in_maps.append({
            "xf": xf.astype(NPBF16),
            "wf": wfull.astype(NPBF16),
            "wdt": wdt.astype(NPBF16),
            "ident": ident,
        })
    return in_maps


def _host_finish(partials):
    """Sum the 8 per-core partial s3 tensors, final squash (the unshard)."""
    s = np.zeros((B, O * D), dtype=np.float64)
    for p in partials:
        s += p.astype(np.float64)
    s = s.reshape(B, O, D)
    n2 = (s * s).sum(axis=-1, keepdims=True)
    n = np.sqrt(n2)
    v = (n2 / (1.0 + n2) / (n + EPS)) * s
    return v.astype(np.float32)


def kernel(x, weight, _trace=False):
    x = np.asarray(x, dtype=np.float32)
    weight = np.asarray(weight, dtype=np.float32)
    if "nc" not in _CACHE:
        _CACHE["nc"] = build()
    nc = _CACHE["nc"]
    in_maps = _host_prep(x, weight)
    res = run_bass_kernel_spmd(
        nc, in_maps, core_ids=list(range(N_CORES)), trace=_trace
    )
    out = _host_finish([res.results[c]["sp"] for c in range(N_CORES)])
    if _trace:
        _CACHE["last_result"] = res
    return out


if __name__ == "__main__":
    rng = np.random.default_rng(0)
    x = rng.standard_normal((B, I_FULL, E)).astype(np.float32)
    w = (0.01 * rng.standard_normal((O, I_FULL, D, E))).astype(np.float32)
    out = kernel(x, w)
    print("out", out.shape, out.dtype, np.abs(out).max())
